# revision 21
# baseline (speedup 1.0000x reference)
"""Multi-head self-attention Trainium2 kernel (8 NeuronCores).

Problem: B=4, S=2048, E=1024, 16 heads x 64 dim, fp32 reference.
    Q = x@Wq+bq; K = x@Wk+bk; V = x@Wv+bv   (weights [in, out])
    attn = softmax(Q K^T / sqrt(64)) V      per (batch, head)
    out  = attn@Wo + bo

Sharding: 8 cores = (batch b, head-half hg). Core c handles batch c//2 and
heads 8*(c%2) .. 8*(c%2)+8 (columns 512*hg .. 512*hg+512 of the QKV
projections, rows 512*hg.. of Wo). Each core produces a partial
[2048, 1024] output contribution; host sums the two half-head partials
per batch and adds bo_eff = bo + bv@Wo (softmax rows sum to 1, so
attn@bv == bv and the V bias can be folded into the output bias on the
host - the kernel never touches bv).

All matmuls run in fp16 (inputs cast on host), accumulating in fp32
PSUM. fp16 carries 10 mantissa bits vs bf16's 8, and every tensor here
is comfortably inside fp16 range, so this quarters the quantization
error for free (validated ~9e-3 total vs the 2e-2 gate).

Per-core dataflow:
  Phase 0: x^T (fp16) loaded once, resident in SBUF.
  Phase 1: K^T [512,2048] and V1 (per-head V columns + ones column for
           softmax sums) from x^T tiles.
  Phase 2 per 512-query block, per head PAIR (even head at partitions
  0-63, odd at 64-127 of the cc slot):
           S^T[k,q] = K @ Q^T with the two heads' matmuls INTERLEAVED:
           they target disjoint PE row-groups (tile_position rows 0 and
           64, auto-derived from base partition), so the hardware runs
           each adjacent pair concurrently (~2x score throughput; the
           64-deep PE window overlaps row-tiled matmuls, see HW row
           tiling).
           exp on ACT (exact, scale=1/8, fp16 out) or DVE (Schraudolph
           fast-exp via fp16 bit pattern: e^(s/8) ~ bitcast_fp16(
           uint16(s*c1 + c2)), ~3% max rel err) - split tuned so both
           engines stay under the PE's per-block budget.
           U^T[q,2,65] = E^T @ V1 accumulated over k-chunks for both
           heads into ONE single-bank PSUM tile; row 64 of each half =
           softmax denominator. One strided reciprocal covers both
           heads; A = U * (1/denom) in fp16, DMA-transposed into A^T.
           Then the output projection A @ Wo per query block, evicted
           fp16 on ACT, stored via gpsimd/scalar DMA queues.
"""

import os
import sys

sys.path.insert(0, "/opt/trn_rl_repo")

import numpy as np

B, S, E = 4, 2048, 1024
H = 8           # heads per core
D = 64          # head dim
HC = 512        # projection columns per core
EC = E // 128   # embed chunks (8)
CC = HC // 128  # col chunks (4)
NB = S // 512   # 512-token blocks (4)
TC = S // 128   # 128-token chunks (16)

# Which (kcp, parity) exp tiles run on DVE fast-exp (rest: exact exp on ACT).
# 7 of 16 per head pair => 28 of 64 per query block.
DVE_TILES = {(1, 0), (1, 1), (3, 0), (4, 0), (4, 1), (6, 0), (6, 1)}

_CACHE = {}
LAST_RESULTS = None


def _build():
    import concourse.bacc as bacc
    import concourse.tile as tile
    from concourse import mybir

    FP32 = mybir.dt.float32
    FP16 = mybir.dt.float16
    U16 = mybir.dt.uint16
    Exp = mybir.ActivationFunctionType.Exp
    # Schraudolph fast-exp in fp16 bit patterns: for y = s*log2(e)/8,
    # 2^y ~= bitcast_fp16(uint16(y*1024 + 15316)). One DVE tensor_scalar
    # (mult+add, fp32->uint16 convert on write) replaces an ACT exp.
    # Max rel err ~3.1% (numerically tuned offset), which softmax
    # normalization largely cancels.
    SCH_C1 = 1024.0 * 1.4426950408889634 / 8.0
    SCH_C2 = 15316.0

    nc = bacc.Bacc("TRN2", target_bir_lowering=False, debug=False,
                   enable_asserts=True, num_devices=8)

    xt_d = nc.dram_tensor("xt", [E, S], FP16, kind="ExternalInput").ap()
    wq_d = nc.dram_tensor("wq", [E, HC], FP16, kind="ExternalInput").ap()
    wk_d = nc.dram_tensor("wk", [E, HC], FP16, kind="ExternalInput").ap()
    wv_d = nc.dram_tensor("wv", [E, HC], FP16, kind="ExternalInput").ap()
    wo_d = nc.dram_tensor("wo", [HC, E], FP16, kind="ExternalInput").ap()
    bq_d = nc.dram_tensor("bq", [HC], FP32, kind="ExternalInput").ap()
    bk_d = nc.dram_tensor("bk", [HC], FP32, kind="ExternalInput").ap()
    out_d = nc.dram_tensor("out", [S, E], FP16, kind="ExternalOutput").ap()

    with tile.TileContext(nc) as tc:
        from contextlib import ExitStack
        with ExitStack() as ctx:
            pers = ctx.enter_context(tc.tile_pool(name="pers", bufs=1))
            qt_pool = ctx.enter_context(tc.tile_pool(name="qt", bufs=2))
            at_pool = ctx.enter_context(tc.tile_pool(name="at", bufs=2))
            e_pool = ctx.enter_context(tc.tile_pool(name="ep", bufs=40))
            r_pool = ctx.enter_context(tc.tile_pool(name="rp", bufs=6))
            a2_pool = ctx.enter_context(tc.tile_pool(name="a2p", bufs=6))
            o_pool = ctx.enter_context(tc.tile_pool(name="op", bufs=3))
            ps_s = ctx.enter_context(
                tc.tile_pool(name="pss", bufs=3, space="PSUM"))
            ps_m = ctx.enter_context(
                tc.tile_pool(name="psm", bufs=2, space="PSUM"))

            # ---- persistent inputs ----
            wq_sb = pers.tile([128, EC, HC], FP16)
            wk_sb = pers.tile([128, EC, HC], FP16)
            wv_sb = pers.tile([128, EC, HC], FP16)
            wo_sb = pers.tile([128, CC, E], FP16)
            xt_sb = pers.tile([128, EC, S], FP16)
            bq_sb = pers.tile([128, CC], FP32)
            bk_sb = pers.tile([128, CC], FP32)
            # Startup DMAs: one big rearrange transfer per tensor/token-block
            # (per-chunk splits pay ~0.6us issue overhead each and serialize;
            # a [128, 8, 512] rearrange is ~3us on one queue). Ordered so
            # K-projection's tb0 inputs (wk + xt block 0) land first.
            xt_r = xt_d.rearrange("(e p) s -> p e s", p=128)
            wk_r = wk_d.rearrange("(e p) h -> p e h", p=128)
            wv_r = wv_d.rearrange("(e p) h -> p e h", p=128)
            wq_r = wq_d.rearrange("(e p) h -> p e h", p=128)
            wo_r = wo_d.rearrange("(d p) e -> p d e", p=128)
            # Round-robin ~256KB pieces over four DMA issue queues so the
            # aggregate DMA bandwidth is engaged (a single queue only
            # reaches a fraction of the 358 GB/s). K-projection's tb0
            # inputs (wk + xt block 0, 2 MB) go first on every queue.
            rr = [nc.sync, nc.scalar, nc.gpsimd]
            qi = [0]

            def ld(dst, src):
                rr[qi[0] % 3].dma_start(dst, src)
                qi[0] += 1

            for ep in range(4):
                ld(xt_sb[:, 2 * ep:2 * ep + 2, 0:512],
                   xt_r[:, 2 * ep:2 * ep + 2, 0:512])
                ld(wk_sb[:, 2 * ep:2 * ep + 2, :],
                   wk_r[:, 2 * ep:2 * ep + 2, :])
            ld(bk_sb[:], bk_d.rearrange("(c p) -> p c", p=128))
            ld(bq_sb[:], bq_d.rearrange("(c p) -> p c", p=128))
            for ep in range(4):
                ld(xt_sb[:, 2 * ep:2 * ep + 2, 512:1024],
                   xt_r[:, 2 * ep:2 * ep + 2, 512:1024])
            for ep in range(2):
                ld(wv_sb[:, 4 * ep:4 * ep + 4, :],
                   wv_r[:, 4 * ep:4 * ep + 4, :])
                ld(wq_sb[:, 4 * ep:4 * ep + 4, :],
                   wq_r[:, 4 * ep:4 * ep + 4, :])
            for tb in (2, 3):
                for ep in range(4):
                    ld(xt_sb[:, 2 * ep:2 * ep + 2,
                             tb * 512:(tb + 1) * 512],
                       xt_r[:, 2 * ep:2 * ep + 2,
                            tb * 512:(tb + 1) * 512])
            for dp in range(2):
                ld(wo_sb[:, 2 * dp:2 * dp + 2, :],
                   wo_r[:, 2 * dp:2 * dp + 2, :])

            kt_sb = pers.tile([128, CC, S], FP16)           # K^T [col, tok]
            v1_sb = pers.tile([128, TC, H, D + 1], FP16)    # V + ones col
            ones_sb = pers.tile([128, 1], FP16)
            nc.vector.memset(ones_sb[:], 1.0)
            nc.vector.tensor_copy(
                v1_sb[:, :, :, D:D + 1],
                ones_sb[:].to_broadcast((128, TC, H, 1)))

            # ---- phase 1: K^T over all token blocks ----
            for tb in range(NB):
                for cc in range(CC):
                    ps = ps_m.tile([128, 512], FP32, tag="m", name="kps")
                    for e in range(EC):
                        nc.tensor.matmul(
                            ps[:],
                            wk_sb[:, e, cc * 128:(cc + 1) * 128],
                            xt_sb[:, e, tb * 512:(tb + 1) * 512],
                            start=(e == 0), stop=(e == EC - 1))
                    nc.vector.tensor_scalar_add(
                        kt_sb[:, cc, tb * 512:(tb + 1) * 512], ps[:],
                        bk_sb[:, cc:cc + 1])

            def v_group(g):
                tb, t = g // 4, g % 4
                ps = ps_m.tile([128, 512], FP32, tag="m", name="vps")
                for e in range(EC):
                    nc.tensor.matmul(
                        ps[:],
                        xt_sb[:, e,
                              tb * 512 + t * 128:tb * 512 + (t + 1) * 128],
                        wv_sb[:, e, :],
                        start=(e == 0), stop=(e == EC - 1))
                nc.vector.tensor_copy(
                    v1_sb[:, tb * 4 + t, :, 0:D],
                    ps[:].rearrange("p (h d) -> p h d", h=H))

            def out_proj_eb(qb, qc, eb):
                at_sb = at_tiles[qb]
                ps = ps_m.tile([128, 512], FP32, tag="m", name="ops")
                for dchunk in range(CC):
                    nc.tensor.matmul(
                        ps[:],
                        at_sb[:, dchunk,
                              qc * 128:(qc + 1) * 128],
                        wo_sb[:, dchunk,
                              eb * 512:(eb + 1) * 512],
                        start=(dchunk == 0), stop=(dchunk == CC - 1))
                o_t = o_pool.tile([128, 512], FP16)
                # evict on ACT, not DVE: a DVE burst here would delay the
                # next pairs' fast-exp tiles and starve their PV matmuls
                nc.scalar.copy(o_t[:], ps[:])
                # alternate output stores across two DMA issue queues so
                # the final drain doesn't serialize on one queue
                eng = nc.gpsimd if eb == 0 else nc.scalar
                eng.dma_start(
                    out_d[qb * 512 + qc * 128:qb * 512 + (qc + 1) * 128,
                          eb * 512:(eb + 1) * 512],
                    o_t[:])

            def q_proj_cc(qb, cc):
                qt_sb = qts[qb]
                ps = ps_m.tile([128, 512], FP32, tag="m", name="qps")
                for e in range(EC):
                    nc.tensor.matmul(
                        ps[:],
                        wq_sb[:, e, cc * 128:(cc + 1) * 128],
                        xt_sb[:, e, qb * 512:(qb + 1) * 512],
                        start=(e == 0), stop=(e == EC - 1))
                nc.vector.tensor_scalar_add(
                    qt_sb[:, cc, :], ps[:], bq_sb[:, cc:cc + 1])

            # ---- phase 2: global software pipeline over 16 (qb, pair)
            # slots. Each pair emits 8 S-steps (4 score matmuls each,
            # even/odd head interleaved so adjacent instructions hit PE
            # row-groups 0/64 and run concurrently, ~2x throughput, plus 2
            # exps). After every other S-step one full PV chain (32
            # small-N matmuls + normalization + transpose) of a ~1.25-pair-
            # lagged chain queue is emitted: fine interleaving keeps the
            # PE duty cycle high inside every HAM activity window (long
            # small-matmul runs re-throttle the clock to 1.2 GHz), and the
            # lag gives ACT/DVE time to finish the exps before the PV
            # matmuls that read them reach the PE queue head. PV chains
            # stay contiguous (u2a/u2b may never straddle another ps_m
            # allocation: pool-rotation order would deadlock). out/q/V
            # projections wedge only at chain boundaries.
            es_slots = {}    # pair slot t -> {parity: [8 exp tiles]}
            at_tiles = {}    # qb -> A^T tile
            qts = {}
            chain_q = []     # pending (slot, qc) chains
            pos = [0]        # global pop-position counter

            def s_step(t, kcp):
                qb, hp = divmod(t, 4)
                qt_sb = qts[qb]
                cc = hp
                es = es_slots.setdefault(t, {0: [], 1: []})
                s_ps = {}
                for par in range(2):
                    s_ps[par] = ps_s.tile([128, 1024], FP32,
                                          tag="s", name=f"sps{par}")
                for half in range(2):
                    kc = 2 * kcp + half
                    for par in range(2):
                        p0 = 64 * par
                        nc.tensor.matmul(
                            s_ps[par][:, half * 512:(half + 1) * 512],
                            kt_sb[p0:p0 + D, cc,
                                  kc * 128:(kc + 1) * 128],
                            qt_sb[p0:p0 + D, cc, :],
                            start=True, stop=True)
                for par in range(2):
                    if (kcp, par) in DVE_TILES:
                        e_raw = e_pool.tile([128, 1024], U16, tag="e",
                                            name="e_raw")
                        nc.vector.tensor_scalar(
                            e_raw[:], s_ps[par][:], SCH_C1, SCH_C2,
                            mybir.AluOpType.mult, mybir.AluOpType.add)
                        es[par].append(e_raw.bitcast(FP16))
                    else:
                        e_t = e_pool.tile([128, 1024], FP16, tag="e",
                                          name="e_t")
                        nc.scalar.activation(e_t[:], s_ps[par][:], Exp,
                                             bias=0.0, scale=0.125)
                        es[par].append(e_t)

            def emit_chain(t, qc):
                """One full PV chain: both heads of pair-slot t, one
                128-query chunk, accumulated over all 16 k-chunks, then
                normalized and DMA-transposed into A^T."""
                qb, hp = divmod(t, 4)
                es = es_slots[t]
                a2 = a2_pool.tile([128, 128], FP16)
                u2a = ps_m.tile([128, D + 1], FP32, tag="m", name="u2a")
                u2b = ps_m.tile([128, D + 1], FP32, tag="m", name="u2b")
                for kcp in range(TC // 2):
                    for h2 in range(2):
                        kc = 2 * kcp + h2
                        sl = slice(h2 * 512 + qc * 128,
                                   h2 * 512 + (qc + 1) * 128)
                        nc.tensor.matmul(
                            u2a[:], es[0][kcp][:, sl],
                            v1_sb[:, kc, 2 * hp, :],
                            start=(kc == 0), stop=(kc == TC - 1))
                        nc.tensor.matmul(
                            u2b[:], es[1][kcp][:, sl],
                            v1_sb[:, kc, 2 * hp + 1, :],
                            start=(kc == 0), stop=(kc == TC - 1))
                for sub, u2 in ((0, u2a), (1, u2b)):
                    r_t = r_pool.tile([128, 1], FP32)
                    nc.vector.reciprocal(r_t[:], u2[:, D:D + 1])
                    nc.vector.tensor_scalar_mul(
                        a2[:, sub * D:(sub + 1) * D], u2[:, 0:D], r_t[:])
                nc.sync.dma_start_transpose(
                    at_tiles[qb][:, hp, qc * 128:(qc + 1) * 128], a2[:])

            out_pending = []

            def pop_chain():
                if not chain_q:
                    return
                t, qc = chain_q.pop(0)
                emit_chain(t, qc)
                # output projection unblocks when the LAST pair (p3) of a
                # query block finishes a q-chunk's chain; emit it half a
                # q-chunk per pop, one pop delayed, so the ~1.1us
                # norm+transpose latency is hidden instead of stalling the
                # out-proj's final dchunk matmul
                if out_pending:
                    out_proj_eb(*out_pending.pop(0))
                if t % 4 == 3:
                    out_pending.append((t // 4, qc, 0))
                    out_pending.append((t // 4, qc, 1))

            qts[0] = qt_pool.tile([128, CC, 512], FP16, name="qt_sb")
            for cc in range(CC):
                q_proj_cc(0, cc)

            for t in range(16):
                qb, hp = divmod(t, 4)
                if hp == 0:
                    at_tiles[qb] = at_pool.tile([128, CC, 512], FP16,
                                                name="at_sb")
                if t == 15:
                    qts[4] = None  # placeholder, never used
                for kcp in range(8):
                    if kcp == 2 and t >= 1:
                        # enqueue previous pair's chains: first pop comes 4+
                        # S-steps after that pair's last exp was emitted
                        chain_q.extend((t - 1, qc) for qc in range(4))
                    s_step(t, kcp)
                    if kcp % 2 == 1:
                        if t == 0:
                            # V projection fills the not-yet-started chain
                            # pipeline: 4 groups per slot, done before the
                            # first chain needs v1
                            for g in range(4):
                                v_group(4 * (kcp // 2) + g)
                        else:
                            pop_chain()
                        if hp == 3 and qb + 1 < NB:
                            if kcp == 1:
                                qts[qb + 1] = qt_pool.tile(
                                    [128, CC, 512], FP16, name="qt_sb")
                            q_proj_cc(qb + 1, (kcp - 1) // 2)
            # drain: remaining chains (incl. the last pair's, enqueued now)
            chain_q.extend((15, qc) for qc in range(4))
            while chain_q:
                pop_chain()
            while out_pending:
                out_proj_eb(*out_pending.pop(0))

    nc.compile()
    return nc


def _register_ntff_hook():
    """The image's antenv lacks axon_hooks, so trace=True would die on the
    import inside run_bass_kernel_spmd. Shim the module and register the
    ctypes NTFF hook from trn_boot when tracing is requested."""
    import types

    if "antenv.axon_hooks" in sys.modules:
        return
    mod = types.ModuleType("antenv.axon_hooks")
    _state = {"hook": None}
    mod.set_axon_ntff_profile_hook = lambda h: _state.__setitem__("hook", h)
    mod.get_axon_ntff_profile_hook = lambda: _state["hook"]
    sys.modules["antenv.axon_hooks"] = mod
    try:
        import antenv

        antenv.axon_hooks = mod
    except ImportError:
        pass
    try:
        from trn_agent_boot.trn_boot import _ntff_profile_via_ctypes

        mod.set_axon_ntff_profile_hook(
            _ntff_profile_via_ctypes("/opt/axon/libaxon_pjrt.so"))
    except Exception:
        pass


def kernel(x, Wq, bq, Wk, bk, Wv, bv, Wo, bo):
    global LAST_RESULTS
    from concourse.bass_utils import run_bass_kernel_spmd

    if "nc" not in _CACHE:
        _CACHE["nc"] = _build()
    nc = _CACHE["nc"]

    f16 = np.float16
    x = np.asarray(x, dtype=np.float32)
    Wq = np.asarray(Wq, dtype=np.float32)
    Wk = np.asarray(Wk, dtype=np.float32)
    Wv = np.asarray(Wv, dtype=np.float32)
    Wo = np.asarray(Wo, dtype=np.float32)
    in_maps = []
    for c in range(8):
        b, hg = c // 2, c % 2
        sl = slice(HC * hg, HC * hg + HC)
        in_maps.append({
            "xt": np.ascontiguousarray(x[b].T).astype(f16),
            "wq": np.ascontiguousarray(Wq[:, sl]).astype(f16),
            "wk": np.ascontiguousarray(Wk[:, sl]).astype(f16),
            "wv": np.ascontiguousarray(Wv[:, sl]).astype(f16),
            "wo": np.ascontiguousarray(Wo[sl, :]).astype(f16),
            "bq": np.ascontiguousarray(np.asarray(bq, dtype=np.float32)[sl]),
            "bk": np.ascontiguousarray(np.asarray(bk, dtype=np.float32)[sl]),
        })

    trace = bool(int(os.environ.get("KERNEL_TRACE", "0")))
    if trace:
        _register_ntff_hook()
    res = run_bass_kernel_spmd(nc, in_maps, list(range(8)), trace=trace)
    LAST_RESULTS = res

    # bv folds into the output bias: softmax rows sum to 1 => attn@bv = bv.
    bo_eff = (np.asarray(bo, dtype=np.float32)
              + np.asarray(bv, dtype=np.float32) @ Wo)
    out = np.empty((B, S, E), dtype=np.float32)
    for b in range(B):
        out[b] = (np.asarray(res.results[2 * b]["out"], dtype=np.float32)
                  + np.asarray(res.results[2 * b + 1]["out"], dtype=np.float32)
                  + bo_eff)
    return out


# revision 23
# speedup vs baseline: 1.1357x; 1.1357x over previous
"""Multi-head self-attention Trainium2 kernel (8 NeuronCores).

Problem: B=4, S=2048, E=1024, 16 heads x 64 dim, fp32 reference.
    Q = x@Wq+bq; K = x@Wk+bk; V = x@Wv+bv   (weights [in, out])
    attn = softmax(Q K^T / sqrt(64)) V      per (batch, head)
    out  = attn@Wo + bo

Sharding: 8 cores = (batch b, head-half hg). Core c handles batch c//2 and
heads 8*(c%2) .. 8*(c%2)+8 (columns 512*hg .. 512*hg+512 of the QKV
projections, rows 512*hg.. of Wo). Each core produces a partial
[2048, 1024] output contribution; host sums the two half-head partials
per batch and adds bo_eff = bo + bv@Wo (softmax rows sum to 1, so
attn@bv == bv and the V bias can be folded into the output bias on the
host - the kernel never touches bv).

All matmuls run in fp16 (inputs cast on host), accumulating in fp32
PSUM. fp16 carries 10 mantissa bits vs bf16's 8, and every tensor here
is comfortably inside fp16 range, so this quarters the quantization
error for free (validated ~9e-3 total vs the 2e-2 gate).

Per-core dataflow:
  Phase 0: x^T (fp16) loaded once, resident in SBUF.
  Phase 1: K^T [512,2048] and V1 (per-head V columns + ones column for
           softmax sums) from x^T tiles.
  Phase 2 per 512-query block, per head PAIR (even head at partitions
  0-63, odd at 64-127 of the cc slot):
           S^T[k,q] = K @ Q^T with the two heads' matmuls INTERLEAVED:
           they target disjoint PE row-groups (tile_position rows 0 and
           64, auto-derived from base partition), so the hardware runs
           each adjacent pair concurrently (~2x score throughput; the
           64-deep PE window overlaps row-tiled matmuls, see HW row
           tiling).
           exp on ACT (exact, scale=1/8, fp16 out) or DVE (Schraudolph
           fast-exp via fp16 bit pattern: e^(s/8) ~ bitcast_fp16(
           uint16(s*c1 + c2)), ~3% max rel err) - split tuned so both
           engines stay under the PE's per-block budget.
           U^T[q,2,65] = E^T @ V1 accumulated over k-chunks for both
           heads into ONE single-bank PSUM tile; row 64 of each half =
           softmax denominator. One strided reciprocal covers both
           heads; A = U * (1/denom) in fp16, DMA-transposed into A^T.
           Then the output projection A @ Wo per query block, evicted
           fp16 on ACT, stored via gpsimd/scalar DMA queues.
"""

import os
import sys

sys.path.insert(0, "/opt/trn_rl_repo")

import numpy as np

B, S, E = 4, 2048, 1024
H = 8           # heads per core
D = 64          # head dim
HC = 512        # projection columns per core
EC = E // 128   # embed chunks (8)
CC = HC // 128  # col chunks (4)
NB = S // 512   # 512-token blocks (4)
TC = S // 128   # 128-token chunks (16)

# Which (kcp, parity) exp tiles run on DVE fast-exp (rest: exact exp on ACT).
# 7 of 16 per head pair => 28 of 64 per query block.
DVE_TILES = {(1, 0), (1, 1), (3, 0), (4, 0), (4, 1), (6, 0), (6, 1)}

_CACHE = {}
LAST_RESULTS = None


def _build():
    import concourse.bacc as bacc
    import concourse.tile as tile
    from concourse import mybir

    FP32 = mybir.dt.float32
    FP16 = mybir.dt.float16
    U16 = mybir.dt.uint16
    Exp = mybir.ActivationFunctionType.Exp
    # Schraudolph fast-exp in fp16 bit patterns: for y = s*log2(e)/8,
    # 2^y ~= bitcast_fp16(uint16(y*1024 + 15316)). One DVE tensor_scalar
    # (mult+add, fp32->uint16 convert on write) replaces an ACT exp.
    # Max rel err ~3.1% (numerically tuned offset), which softmax
    # normalization largely cancels.
    SCH_C1 = 1024.0 * 1.4426950408889634 / 8.0
    SCH_C2 = 15316.0

    nc = bacc.Bacc("TRN2", target_bir_lowering=False, debug=False,
                   enable_asserts=True, num_devices=8)

    xt_d = nc.dram_tensor("xt", [E, S], FP16, kind="ExternalInput").ap()
    wq_d = nc.dram_tensor("wq", [E, HC], FP16, kind="ExternalInput").ap()
    wk_d = nc.dram_tensor("wk", [E, HC], FP16, kind="ExternalInput").ap()
    wv_d = nc.dram_tensor("wv", [E, HC], FP16, kind="ExternalInput").ap()
    wo_d = nc.dram_tensor("wo", [HC, E], FP16, kind="ExternalInput").ap()
    bq_d = nc.dram_tensor("bq", [HC], FP32, kind="ExternalInput").ap()
    bk_d = nc.dram_tensor("bk", [HC], FP32, kind="ExternalInput").ap()
    out_d = nc.dram_tensor("out", [S, E], FP16, kind="ExternalOutput").ap()

    with tile.TileContext(nc) as tc:
        from contextlib import ExitStack
        with ExitStack() as ctx:
            pers = ctx.enter_context(tc.tile_pool(name="pers", bufs=1))
            qt_pool = ctx.enter_context(tc.tile_pool(name="qt", bufs=2))
            at_pool = ctx.enter_context(tc.tile_pool(name="at", bufs=2))
            e_pool = ctx.enter_context(tc.tile_pool(name="ep", bufs=40))
            r_pool = ctx.enter_context(tc.tile_pool(name="rp", bufs=6))
            a2_pool = ctx.enter_context(tc.tile_pool(name="a2p", bufs=6))
            o_pool = ctx.enter_context(tc.tile_pool(name="op", bufs=3))
            ps_s = ctx.enter_context(
                tc.tile_pool(name="pss", bufs=3, space="PSUM"))
            ps_m = ctx.enter_context(
                tc.tile_pool(name="psm", bufs=2, space="PSUM"))

            # ---- persistent inputs ----
            wq_sb = pers.tile([128, EC, HC], FP16)
            wk_sb = pers.tile([128, EC, HC], FP16)
            wv_sb = pers.tile([128, EC, HC], FP16)
            wo_sb = pers.tile([128, CC, E], FP16)
            xt_sb = pers.tile([128, EC, S], FP16)
            bq_sb = pers.tile([128, CC], FP32)
            bk_sb = pers.tile([128, CC], FP32)
            # Startup DMAs: one big rearrange transfer per tensor/token-block
            # (per-chunk splits pay ~0.6us issue overhead each and serialize;
            # a [128, 8, 512] rearrange is ~3us on one queue). Ordered so
            # K-projection's tb0 inputs (wk + xt block 0) land first.
            xt_r = xt_d.rearrange("(e p) s -> p e s", p=128)
            wk_r = wk_d.rearrange("(e p) h -> p e h", p=128)
            wv_r = wv_d.rearrange("(e p) h -> p e h", p=128)
            wq_r = wq_d.rearrange("(e p) h -> p e h", p=128)
            wo_r = wo_d.rearrange("(d p) e -> p d e", p=128)
            # Round-robin ~256KB pieces over four DMA issue queues so the
            # aggregate DMA bandwidth is engaged (a single queue only
            # reaches a fraction of the 358 GB/s). K-projection's tb0
            # inputs (wk + xt block 0, 2 MB) go first on every queue.
            rr = [nc.sync, nc.scalar, nc.gpsimd]
            qi = [0]

            def ld(dst, src):
                rr[qi[0] % 3].dma_start(dst, src)
                qi[0] += 1

            for ep in range(4):
                ld(xt_sb[:, 2 * ep:2 * ep + 2, 0:512],
                   xt_r[:, 2 * ep:2 * ep + 2, 0:512])
                ld(wk_sb[:, 2 * ep:2 * ep + 2, :],
                   wk_r[:, 2 * ep:2 * ep + 2, :])
            ld(bk_sb[:], bk_d.rearrange("(c p) -> p c", p=128))
            ld(bq_sb[:], bq_d.rearrange("(c p) -> p c", p=128))
            for ep in range(4):
                ld(xt_sb[:, 2 * ep:2 * ep + 2, 512:1024],
                   xt_r[:, 2 * ep:2 * ep + 2, 512:1024])
            for ep in range(2):
                ld(wv_sb[:, 4 * ep:4 * ep + 4, :],
                   wv_r[:, 4 * ep:4 * ep + 4, :])
                ld(wq_sb[:, 4 * ep:4 * ep + 4, :],
                   wq_r[:, 4 * ep:4 * ep + 4, :])
            for tb in (2, 3):
                for ep in range(4):
                    ld(xt_sb[:, 2 * ep:2 * ep + 2,
                             tb * 512:(tb + 1) * 512],
                       xt_r[:, 2 * ep:2 * ep + 2,
                            tb * 512:(tb + 1) * 512])
            for dp in range(2):
                ld(wo_sb[:, 2 * dp:2 * dp + 2, :],
                   wo_r[:, 2 * dp:2 * dp + 2, :])

            kt_sb = pers.tile([128, CC, S], FP16)           # K^T [col, tok]
            v1_sb = pers.tile([128, TC, H, D + 1], FP16)    # V + ones col
            ones_sb = pers.tile([128, 1], FP16)
            nc.vector.memset(ones_sb[:], 1.0)
            nc.vector.tensor_copy(
                v1_sb[:, :, :, D:D + 1],
                ones_sb[:].to_broadcast((128, TC, H, 1)))

            # ---- phase 1: K^T over all token blocks ----
            for tb in range(NB):
                for cc in range(CC):
                    ps = ps_m.tile([128, 512], FP32, tag="m", name="kps")
                    for e in range(EC):
                        nc.tensor.matmul(
                            ps[:],
                            wk_sb[:, e, cc * 128:(cc + 1) * 128],
                            xt_sb[:, e, tb * 512:(tb + 1) * 512],
                            start=(e == 0), stop=(e == EC - 1))
                    nc.vector.tensor_scalar_add(
                        kt_sb[:, cc, tb * 512:(tb + 1) * 512], ps[:],
                        bk_sb[:, cc:cc + 1])

            def v_group(g):
                tb, t = g // 4, g % 4
                ps = ps_m.tile([128, 512], FP32, tag="m", name="vps")
                for e in range(EC):
                    nc.tensor.matmul(
                        ps[:],
                        xt_sb[:, e,
                              tb * 512 + t * 128:tb * 512 + (t + 1) * 128],
                        wv_sb[:, e, :],
                        start=(e == 0), stop=(e == EC - 1))
                nc.vector.tensor_copy(
                    v1_sb[:, tb * 4 + t, :, 0:D],
                    ps[:].rearrange("p (h d) -> p h d", h=H))

            def out_proj_eb(qb, qc, eb):
                at_sb = at_tiles[qb]
                ps = ps_m.tile([128, 512], FP32, tag="m", name="ops")
                for dchunk in range(CC):
                    nc.tensor.matmul(
                        ps[:],
                        at_sb[:, dchunk,
                              qc * 128:(qc + 1) * 128],
                        wo_sb[:, dchunk,
                              eb * 512:(eb + 1) * 512],
                        start=(dchunk == 0), stop=(dchunk == CC - 1))
                o_t = o_pool.tile([128, 512], FP16)
                # evict on ACT, not DVE: a DVE burst here would delay the
                # next pairs' fast-exp tiles and starve their PV matmuls
                nc.scalar.copy(o_t[:], ps[:])
                # alternate output stores across two DMA issue queues so
                # the final drain doesn't serialize on one queue
                eng = nc.gpsimd if eb == 0 else nc.scalar
                eng.dma_start(
                    out_d[qb * 512 + qc * 128:qb * 512 + (qc + 1) * 128,
                          eb * 512:(eb + 1) * 512],
                    o_t[:])

            def q_proj_cc(qb, cc):
                qt_sb = qts[qb]
                ps = ps_m.tile([128, 512], FP32, tag="m", name="qps")
                for e in range(EC):
                    nc.tensor.matmul(
                        ps[:],
                        wq_sb[:, e, cc * 128:(cc + 1) * 128],
                        xt_sb[:, e, qb * 512:(qb + 1) * 512],
                        start=(e == 0), stop=(e == EC - 1))
                nc.vector.tensor_scalar_add(
                    qt_sb[:, cc, :], ps[:], bq_sb[:, cc:cc + 1])

            # ---- phase 2: global software pipeline over 16 (qb, pair)
            # slots. Each pair emits 8 S-steps (4 score matmuls each,
            # even/odd head interleaved so adjacent instructions hit PE
            # row-groups 0/64 and run concurrently, ~2x throughput, plus 2
            # exps). After every other S-step one full PV chain (32
            # small-N matmuls + normalization + transpose) of a ~1.25-pair-
            # lagged chain queue is emitted: fine interleaving keeps the
            # PE duty cycle high inside every HAM activity window (long
            # small-matmul runs re-throttle the clock to 1.2 GHz), and the
            # lag gives ACT/DVE time to finish the exps before the PV
            # matmuls that read them reach the PE queue head. PV chains
            # stay contiguous (u2a/u2b may never straddle another ps_m
            # allocation: pool-rotation order would deadlock). out/q/V
            # projections wedge only at chain boundaries.
            es_slots = {}    # pair slot t -> {parity: [8 exp tiles]}
            at_tiles = {}    # qb -> A^T tile
            qts = {}
            chain_q = []     # pending (slot, qc) chains
            pos = [0]        # global pop-position counter

            def s_step(t, kcp):
                qb, hp = divmod(t, 4)
                qt_sb = qts[qb]
                cc = hp
                es = es_slots.setdefault(t, {0: [], 1: []})
                s_ps = {}
                for par in range(2):
                    s_ps[par] = ps_s.tile([128, 1024], FP32,
                                          tag="s", name=f"sps{par}")
                for half in range(2):
                    kc = 2 * kcp + half
                    for par in range(2):
                        p0 = 64 * par
                        nc.tensor.matmul(
                            s_ps[par][:, half * 512:(half + 1) * 512],
                            kt_sb[p0:p0 + D, cc,
                                  kc * 128:(kc + 1) * 128],
                            qt_sb[p0:p0 + D, cc, :],
                            start=True, stop=True)
                for par in range(2):
                    if (kcp, par) in DVE_TILES:
                        e_raw = e_pool.tile([128, 1024], U16, tag="e",
                                            name="e_raw")
                        nc.vector.tensor_scalar(
                            e_raw[:], s_ps[par][:], SCH_C1, SCH_C2,
                            mybir.AluOpType.mult, mybir.AluOpType.add)
                        es[par].append(e_raw.bitcast(FP16))
                    else:
                        e_t = e_pool.tile([128, 1024], FP16, tag="e",
                                          name="e_t")
                        nc.scalar.activation(e_t[:], s_ps[par][:], Exp,
                                             bias=0.0, scale=0.125)
                        es[par].append(e_t)

            def emit_chain(t, qc):
                """One full PV chain: both heads of pair-slot t, one
                128-query chunk, accumulated over all 16 k-chunks, then
                normalized and DMA-transposed into A^T."""
                qb, hp = divmod(t, 4)
                es = es_slots[t]
                a2 = a2_pool.tile([128, 128], FP16)
                u2a = ps_m.tile([128, D + 1], FP32, tag="m", name="u2a")
                u2b = ps_m.tile([128, D + 1], FP32, tag="m", name="u2b")
                for kcp in range(TC // 2):
                    for h2 in range(2):
                        kc = 2 * kcp + h2
                        sl = slice(h2 * 512 + qc * 128,
                                   h2 * 512 + (qc + 1) * 128)
                        nc.tensor.matmul(
                            u2a[:], es[0][kcp][:, sl],
                            v1_sb[:, kc, 2 * hp, :],
                            start=(kc == 0), stop=(kc == TC - 1))
                        nc.tensor.matmul(
                            u2b[:], es[1][kcp][:, sl],
                            v1_sb[:, kc, 2 * hp + 1, :],
                            start=(kc == 0), stop=(kc == TC - 1))
                for sub, u2 in ((0, u2a), (1, u2b)):
                    r_t = r_pool.tile([128, 1], FP32)
                    nc.vector.reciprocal(r_t[:], u2[:, D:D + 1])
                    nc.vector.tensor_scalar_mul(
                        a2[:, sub * D:(sub + 1) * D], u2[:, 0:D], r_t[:])
                nc.sync.dma_start_transpose(
                    at_tiles[qb][:, hp, qc * 128:(qc + 1) * 128], a2[:])

            out_pending = []

            def pop_chain():
                if not chain_q:
                    return
                t, qc = chain_q.pop(0)
                emit_chain(t, qc)
                # output projection unblocks when the LAST pair (p3) of a
                # query block finishes a q-chunk's chain; emit it half a
                # q-chunk per pop, one pop delayed, so the ~1.1us
                # norm+transpose latency is hidden instead of stalling the
                # out-proj's final dchunk matmul
                if out_pending:
                    qb_o, qc_o = out_pending.pop(0)
                    out_proj_eb(qb_o, qc_o, 0)
                    out_proj_eb(qb_o, qc_o, 1)
                if t % 4 == 3:
                    out_pending.append((t // 4, qc))

            qts[0] = qt_pool.tile([128, CC, 512], FP16, name="qt_sb")
            for cc in range(CC):
                q_proj_cc(0, cc)

            for t in range(16):
                qb, hp = divmod(t, 4)
                if hp == 0:
                    at_tiles[qb] = at_pool.tile([128, CC, 512], FP16,
                                                name="at_sb")
                if t == 15:
                    qts[4] = None  # placeholder, never used
                for kcp in range(8):
                    if kcp == 2 and t >= 1:
                        # enqueue previous pair's chains: first pop comes 4+
                        # S-steps after that pair's last exp was emitted
                        chain_q.extend((t - 1, qc) for qc in range(4))
                    s_step(t, kcp)
                    if kcp % 2 == 1:
                        if t == 0:
                            # V projection fills the not-yet-started chain
                            # pipeline: 4 groups per slot, done before the
                            # first chain needs v1
                            for g in range(4):
                                v_group(4 * (kcp // 2) + g)
                        else:
                            pop_chain()
                        if hp == 3 and qb + 1 < NB:
                            if kcp == 1:
                                qts[qb + 1] = qt_pool.tile(
                                    [128, CC, 512], FP16, name="qt_sb")
                            q_proj_cc(qb + 1, (kcp - 1) // 2)
            # drain: remaining chains (incl. the last pair's, enqueued now)
            chain_q.extend((15, qc) for qc in range(4))
            while chain_q:
                pop_chain()
            while out_pending:
                qb_o, qc_o = out_pending.pop(0)
                out_proj_eb(qb_o, qc_o, 0)
                out_proj_eb(qb_o, qc_o, 1)

    nc.compile()
    return nc


def _register_ntff_hook():
    """The image's antenv lacks axon_hooks, so trace=True would die on the
    import inside run_bass_kernel_spmd. Shim the module and register the
    ctypes NTFF hook from trn_boot when tracing is requested."""
    import types

    if "antenv.axon_hooks" in sys.modules:
        return
    mod = types.ModuleType("antenv.axon_hooks")
    _state = {"hook": None}
    mod.set_axon_ntff_profile_hook = lambda h: _state.__setitem__("hook", h)
    mod.get_axon_ntff_profile_hook = lambda: _state["hook"]
    sys.modules["antenv.axon_hooks"] = mod
    try:
        import antenv

        antenv.axon_hooks = mod
    except ImportError:
        pass
    try:
        from trn_agent_boot.trn_boot import _ntff_profile_via_ctypes

        mod.set_axon_ntff_profile_hook(
            _ntff_profile_via_ctypes("/opt/axon/libaxon_pjrt.so"))
    except Exception:
        pass


def kernel(x, Wq, bq, Wk, bk, Wv, bv, Wo, bo):
    global LAST_RESULTS
    from concourse.bass_utils import run_bass_kernel_spmd

    if "nc" not in _CACHE:
        _CACHE["nc"] = _build()
    nc = _CACHE["nc"]

    f16 = np.float16
    x = np.asarray(x, dtype=np.float32)
    Wq = np.asarray(Wq, dtype=np.float32)
    Wk = np.asarray(Wk, dtype=np.float32)
    Wv = np.asarray(Wv, dtype=np.float32)
    Wo = np.asarray(Wo, dtype=np.float32)
    in_maps = []
    for c in range(8):
        b, hg = c // 2, c % 2
        sl = slice(HC * hg, HC * hg + HC)
        in_maps.append({
            "xt": np.ascontiguousarray(x[b].T).astype(f16),
            "wq": np.ascontiguousarray(Wq[:, sl]).astype(f16),
            "wk": np.ascontiguousarray(Wk[:, sl]).astype(f16),
            "wv": np.ascontiguousarray(Wv[:, sl]).astype(f16),
            "wo": np.ascontiguousarray(Wo[sl, :]).astype(f16),
            "bq": np.ascontiguousarray(np.asarray(bq, dtype=np.float32)[sl]),
            "bk": np.ascontiguousarray(np.asarray(bk, dtype=np.float32)[sl]),
        })

    trace = bool(int(os.environ.get("KERNEL_TRACE", "0")))
    if trace:
        _register_ntff_hook()
    res = run_bass_kernel_spmd(nc, in_maps, list(range(8)), trace=trace)
    LAST_RESULTS = res

    # bv folds into the output bias: softmax rows sum to 1 => attn@bv = bv.
    bo_eff = (np.asarray(bo, dtype=np.float32)
              + np.asarray(bv, dtype=np.float32) @ Wo)
    out = np.empty((B, S, E), dtype=np.float32)
    for b in range(B):
        out[b] = (np.asarray(res.results[2 * b]["out"], dtype=np.float32)
                  + np.asarray(res.results[2 * b + 1]["out"], dtype=np.float32)
                  + bo_eff)
    return out
